# revision 45
# baseline (speedup 1.0000x reference)
"""Trainium2 Bass kernel: LayerNorm -> top-1 MoE (host-routed, expert-sorted
slots, bf16 matmuls) -> v = clip(moe @ proj_w + proj_b, +-3) -> tridiagonal
Green's-function diagonal via chunked Mobius scan -> out = moe + bk*spec_out.

Sharding: core c = 2b + h owns row b's tokens routed to experts {2h, 2h+1}.
Tokens are expert-sorted into 128-slot tiles on the host (routing replicated
with jax CPU ops exactly as the reference computes it); each core evaluates
only its two experts on its slot tiles.  v values are pair-AllGathered in slot
order and reordered to sequence order on-device via grouped indirect_copy +
PE-transpose; the scan then runs as before and G is scattered back to slot
order the same way.  The host inverts the slot permutation on the output rows.
"""
import numpy as np
import concourse.bacc as bacc
import concourse.mybir as mybir
from concourse.tile import TileContext
from concourse.bass_utils import run_bass_kernel_spmd
from concourse.alu_op_type import AluOpType

F32 = mybir.dt.float32
BF16 = mybir.dt.bfloat16
U16 = mybir.dt.uint16
AF = mybir.ActivationFunctionType
AX = mybir.AxisListType
MULT, ADD, SUB = AluOpType.mult, AluOpType.add, AluOpType.subtract
MAXOP, MINOP = AluOpType.max, AluOpType.min

B, N, D, E = 4, 2048, 512, 4
H = 4 * D
P = 128
NCORE = 8


def build(proj_b_imm, capt, debug=False):
    TT = 2 * capt          # token tiles per core
    S = TT * P             # slots per core
    S2 = capt * P          # slots per expert
    nc = bacc.Bacc()
    dt = nc.dram_tensor
    xs = dt("xs", [S, D], BF16, kind="ExternalInput")
    gamT = dt("gamT", [P, 4], F32, kind="ExternalInput")
    betT = dt("betT", [P, 4], F32, kind="ExternalInput")
    gwsb = dt("gwsb", [P, 16], BF16, kind="ExternalInput")
    gatebb = dt("gatebb", [P, E], F32, kind="ExternalInput")
    w1s = dt("w1s", [2 * D, H], BF16, kind="ExternalInput")
    b1s = dt("b1s", [P, 32], F32, kind="ExternalInput")
    w2s = dt("w2s", [2 * H, D], BF16, kind="ExternalInput")
    b2s = dt("b2s", [P, 2 * D], F32, kind="ExternalInput")
    projwb = dt("projwb", [P, D], F32, kind="ExternalInput")
    w0b = dt("w0b", [P, D], F32, kind="ExternalInput")
    w1ob = dt("w1ob", [P, D], F32, kind="ExternalInput")
    outbb = dt("outbb", [P, D], F32, kind="ExternalInput")
    shmat = dt("shmat", [P, 7 * P], F32, kind="ExternalInput")
    jmat = dt("jmat", [P, P], F32, kind="ExternalInput")
    idm = dt("idm", [P, P], F32, kind="ExternalInput")
    vidxd = dt("vidxd", [P, 16], U16, kind="ExternalInput")
    gidxd = dt("gidxd", [P, 24], U16, kind="ExternalInput")
    out = dt("out", [S, D], F32, kind="ExternalOutput")
    if debug:
        vdbg = dt("vdbg", [S], F32, kind="ExternalOutput")
        avdbg = dt("avdbg", [P, 16], F32, kind="ExternalOutput")
        grdbg = dt("grdbg", [N], F32, kind="ExternalOutput")
        gidbg = dt("gidbg", [N], F32, kind="ExternalOutput")
        grsdbg = dt("grsdbg", [P, TT], F32, kind="ExternalOutput")
        gisdbg = dt("gisdbg", [P, TT], F32, kind="ExternalOutput")
        moedbg = dt("moedbg", [S, D], F32, kind="ExternalOutput")

    with TileContext(nc) as tc:
        with (tc.tile_pool(name="cst", bufs=1) as cst,
              tc.tile_pool(name="big", bufs=1) as big,
              tc.tile_pool(name="ln", bufs=3) as ln,
              tc.tile_pool(name="rot", bufs=4) as rot,
              tc.tile_pool(name="sml", bufs=1) as sml,
              tc.tile_pool(name="ps", bufs=8, space="PSUM") as psp,
              tc.tile_pool(name="dr", bufs=1, space="DRAM") as dr):

            def tt(o, a, b, op):
                nc.vector.tensor_tensor(out=o, in0=a, in1=b, op=op)

            def ts(o, a, s1, s2, op0, op1=None):
                if op1 is None:
                    nc.vector.tensor_scalar(out=o, in0=a, scalar1=s1,
                                            scalar2=None, op0=op0)
                else:
                    nc.vector.tensor_scalar(out=o, in0=a, scalar1=s1,
                                            scalar2=s2, op0=op0, op1=op1)

            def stt(o, a, s, b, op0, op1):
                nc.vector.scalar_tensor_tensor(out=o, in0=a, scalar=s, in1=b,
                                               op0=op0, op1=op1)

            def cp(o, a):
                nc.vector.tensor_copy(out=o, in_=a)

            # ---- constants to SBUF ----
            gamt = cst.tile([P, 4], F32, tag="gamt")
            bett = cst.tile([P, 4], F32, tag="bett")
            gwst = cst.tile([P, 16], BF16, tag="gwst")
            gatebt = cst.tile([P, E], F32, tag="gatebt")
            b1t = cst.tile([P, 32], F32, tag="b1t")
            b2t = cst.tile([P, 2 * D], F32, tag="b2t")
            projwt = cst.tile([P, D], F32, tag="projwt")
            w0t = cst.tile([P, D], F32, tag="w0t")
            w1ot = cst.tile([P, D], F32, tag="w1ot")
            outbt = cst.tile([P, D], F32, tag="outbt")
            sht = cst.tile([P, 7 * P], F32, tag="sht")
            jmt = cst.tile([P, P], F32, tag="jmt")
            idt = cst.tile([P, P], F32, tag="idt")
            vidx = cst.tile([P, 16], U16, tag="vidx")
            gidx = cst.tile([P, 24], U16, tag="gidx")
            for tl, src in ((gamt, gamT), (bett, betT), (gwst, gwsb),
                            (gatebt, gatebb), (b1t, b1s), (b2t, b2s),
                            (projwt, projwb), (w0t, w0b), (w1ot, w1ob),
                            (outbt, outbb), (sht, shmat), (jmt, jmat),
                            (idt, idm), (vidx, vidxd), (gidx, gidxd)):
                nc.sync.dma_start(out=tl, in_=src[:])

            xnT = big.tile([P, 4 * S], BF16, tag="xnT")     # xn^T bf16
            hT = big.tile([P, 16 * S2], BF16, tag="hT")     # h^T (per hc; reused across el)
            moe = big.tile([P, TT * D], F32, tag="moe")
            pm = sml.tile([P, TT], F32, tag="pm")
            # resident weights: w1sb[(el,db)] rows d -> 16 hc x 128 h cols
            # (DMAs issued after stage A so the x tiles go out first)
            w1sb = big.tile([P, 8 * H], BF16, tag="w1sb")
            w2sb = big.tile([P, 32 * D], BF16, tag="w2sb")

            # DRAM bounce buffers
            vlocA = dr.tile([S2], F32, name="vlocA", tag="vlocA")
            vlocB = dr.tile([S2], F32, name="vlocB", tag="vlocB")
            vrowA = dr.tile([2 * S2], F32, name="vrowA", tag="vrowA")
            vrowB = dr.tile([2 * S2], F32, name="vrowB", tag="vrowB")
            grd = dr.tile([N], F32, name="grd", tag="grd")
            gid = dr.tile([N], F32, name="gid", tag="gid")
            rep = big.tile([P, max(4 * S2, 2 * N)], F32, tag="rep")

            # ================= stage A: LN + transpose + gate =================
            for tb in range(TT):
                xt = ln.tile([P, D], BF16)
                nc.sync.dma_start(out=xt, in_=xs[tb * P:(tb + 1) * P, :])
                musum = sml.tile([P, 1], F32, tag="musum")
                nc.vector.tensor_reduce(out=musum, in_=xt, axis=AX.X, op=ADD)
                mu = sml.tile([P, 1], F32, tag="mu")
                ts(mu, musum, 1.0 / D, None, MULT)
                scr = ln.tile([P, D], F32)
                sqsum = sml.tile([P, 1], F32, tag="sqsum")
                nc.scalar.activation(out=scr, in_=xt, func=AF.Square,
                                     accum_out=sqsum[:, 0:1])
                mu2 = sml.tile([P, 1], F32, tag="mu2")
                tt(mu2, mu, mu, MULT)
                vtmp = sml.tile([P, 1], F32, tag="vtmp")
                ts(vtmp, sqsum, 1.0 / D, 1e-5, MULT, ADD)
                var_ = sml.tile([P, 1], F32, tag="var_")
                tt(var_, vtmp, mu2, SUB)               # E[x^2] - mu^2 + eps
                vsq = sml.tile([P, 1], F32, tag="vsq")
                nc.scalar.activation(out=vsq, in_=var_, func=AF.Sqrt)
                rstd = sml.tile([P, 1], F32, tag="rstd")
                nc.vector.reciprocal(out=rstd, in_=vsq)
                nmu = sml.tile([P, 1], F32, tag="nmu")
                ts(nmu, mu, -1.0, None, MULT)
                nmurs = sml.tile([P, 1], F32, tag="nmurs")
                tt(nmurs, nmu, rstd, MULT)
                xnp = ln.tile([P, D], F32)
                ts(xnp, xt, rstd[:, 0:1], nmurs[:, 0:1], MULT, ADD)
                for db in range(4):
                    pst = psp.tile([P, P], F32, tag="mm")
                    nc.tensor.transpose(pst[:], xnp[:, db * P:(db + 1) * P], idt[:])
                    ts(xnT[:, db * S + tb * P: db * S + (tb + 1) * P], pst,
                       gamt[:, db:db + 1], bett[:, db:db + 1], MULT, ADD)
                # gate -> top prob (bf16; routing itself fixed by host layout)
                psg = psp.tile([P, E], F32, tag="mm")
                for db in range(4):
                    nc.tensor.matmul(out=psg,
                                     lhsT=xnT[:, db * S + tb * P: db * S + (tb + 1) * P],
                                     rhs=gwst[:, db * E:(db + 1) * E],
                                     start=(db == 0), stop=(db == 3))
                lg = sml.tile([P, E], F32, tag="lg")
                stt(lg, psg, 1.0, gatebt, MULT, ADD)
                mx = sml.tile([P, 1], F32, tag="mx")
                nc.vector.tensor_reduce(out=mx, in_=lg, axis=AX.X, op=MAXOP)
                mneg = sml.tile([P, 1], F32, tag="mneg")
                ts(mneg, mx, -1.0, None, MULT)
                el_ = sml.tile([P, E], F32, tag="el")
                ssum = sml.tile([P, 1], F32, tag="ssum")
                nc.scalar.activation(out=el_, in_=lg, func=AF.Exp,
                                     bias=mneg[:, 0:1], scale=1.0,
                                     accum_out=ssum[:, 0:1])
                nc.vector.reciprocal(out=pm[:, tb:tb + 1], in_=ssum)

            # ================= stage B: routed MoE (2 experts/core) ===========
            for el in range(2):
                for db in range(4):
                    nc.sync.dma_start(
                        out=w1sb[:, (el * 4 + db) * H:(el * 4 + db + 1) * H],
                        in_=w1s[el * D + db * P: el * D + (db + 1) * P, :])
            for el in range(2):
                for hc in range(16):
                    nc.sync.dma_start(
                        out=w2sb[:, (el * 16 + hc) * D:(el * 16 + hc + 1) * D],
                        in_=w2s[el * H + hc * P: el * H + (hc + 1) * P, :])
            pieces = [(off, min(512, S2 - off)) for off in range(0, S2, 512)]
            for el in range(2):
                for hc in range(16):
                    phs = [psp.tile([P, w], F32, tag="mm", name=f"phs{pi}")
                           for pi, (off, w) in enumerate(pieces)]
                    for db in range(4):
                        for pi, (off, w) in enumerate(pieces):
                            nc.tensor.matmul(
                                out=phs[pi],
                                lhsT=w1sb[:, (el * 4 + db) * H + hc * P:
                                          (el * 4 + db) * H + (hc + 1) * P],
                                rhs=xnT[:, db * S + el * S2 + off:
                                        db * S + el * S2 + off + w],
                                start=(db == 0), stop=(db == 3))
                    for pi, (off, w) in enumerate(pieces):
                        nc.scalar.activation(
                            out=hT[:, hc * S2 + off: hc * S2 + off + w],
                            in_=phs[pi], func=AF.Gelu_apprx_tanh,
                            bias=b1t[:, el * 16 + hc: el * 16 + hc + 1],
                            scale=1.0)
                for grp0 in range(0, capt, 5):
                    grp = range(grp0, min(grp0 + 5, capt))
                    pso = {tb: psp.tile([P, D], F32, tag="mm", name=f"pso{el}_{tb}")
                           for tb in grp}
                    for hc in range(16):
                        for tb in grp:
                            nc.tensor.matmul(
                                out=pso[tb],
                                lhsT=hT[:, hc * S2 + tb * P:
                                        hc * S2 + (tb + 1) * P],
                                rhs=w2sb[:, (el * 16 + hc) * D:
                                         (el * 16 + hc + 1) * D],
                                start=(hc == 0), stop=(hc == 15))
                    for tb in grp:
                        tg = el * capt + tb
                        eo = rot.tile([P, D], F32, tag="wrk")
                        stt(eo, pso[tb], 1.0, b2t[:, el * D:(el + 1) * D],
                            MULT, ADD)
                        ts(moe[:, tg * D:(tg + 1) * D], eo, pm[:, tg:tg + 1],
                           None, MULT)
                # v = clip(moe@proj_w + proj_b) for this expert's tiles,
                # then pair-AllGather this half right away (overlaps the
                # other expert's compute)
                vloc_el = vlocA if el == 0 else vlocB
                for tb in range(capt):
                    tg = el * capt + tb
                    tv = rot.tile([P, D], F32, tag="wrk")
                    tt(tv, moe[:, tg * D:(tg + 1) * D], projwt, MULT)
                    vs = sml.tile([P, 1], F32, tag="vs")
                    nc.vector.tensor_reduce(out=vs, in_=tv, axis=AX.X, op=ADD)
                    vt1 = sml.tile([P, 1], F32, tag="vt1")
                    ts(vt1, vs, proj_b_imm, 3.0, ADD, MINOP)
                    vt2 = sml.tile([P, 1], F32, tag="vt2")
                    ts(vt2, vt1, -3.0, None, MAXOP)
                    nc.sync.dma_start(out=vloc_el[tb * P:(tb + 1) * P],
                                      in_=vt2[:, 0:1])
                    if debug:
                        nc.sync.dma_start(out=vdbg[tg * P:(tg + 1) * P],
                                          in_=vt2[:, 0:1])
                nc.gpsimd.collective_compute(
                    "AllGather", AluOpType.bypass,
                    replica_groups=[[0, 1], [2, 3], [4, 5], [6, 7]],
                    ins=[vloc_el.opt()],
                    outs=[(vrowA if el == 0 else vrowB).opt()])

            # ================= stage D: broadcast + reorder to seq ===========
            v_rep = rep[:, 0:4 * S2]
            nc.sync.dma_start(out=v_rep[:, 0:2 * S2],
                              in_=vrowA[0:2 * S2].partition_broadcast(P))
            nc.sync.dma_start(out=v_rep[:, 2 * S2:4 * S2],
                              in_=vrowB[0:2 * S2].partition_broadcast(P))

            # stage C/D are folded into stage B per expert (see below)
            av = sml.tile([P, 16], F32, tag="av")
            for c in range(2):
                mid = sml.tile([P, P], F32, tag=f"vmid{c}", name=f"vmid{c}")
                nc.gpsimd.indirect_copy(
                    out=mid[:], data=v_rep, idxs=vidx[:, 8 * c:8 * (c + 1)],
                    i_know_ap_gather_is_preferred=True)
                pst = psp.tile([P, P], F32, tag="mm")
                nc.tensor.transpose(pst[:], mid[:], idt[:])
                cp(av[:, 8 * c:8 * (c + 1)], pst[:, 0:P:16])
            if debug:
                nc.sync.dma_start(out=avdbg[:], in_=av)
            arf = sml.tile([P, 16], F32, tag="arf")
            ts(arf, av, -1.0, 2.0, MULT, ADD)        # a_re = 2 - v
            psj = psp.tile([P, 16], F32, tag="mm")
            nc.tensor.matmul(out=psj, lhsT=jmt[:], rhs=arf[:], start=True, stop=True)
            arb = sml.tile([P, 16], F32, tag="arb")
            cp(arb, psj[:, 15::-1])                  # a_re reversed seq, chunk-major
            adup = sml.tile([P, 64], F32, tag="adup")
            cp(adup[:, 0:64:4], arf)
            cp(adup[:, 1:64:4], arb)
            cp(adup[:, 2:64:4], arf)
            cp(adup[:, 3:64:4], arb)

            # ================= stage E: Mobius scan ===========================
            # L1: chunk transfer matrices, cols (f0, b0, f1, b1)
            sr = [sml.tile([P, 4], F32, tag=f"l1sr{i}", name=f"l1sr{i}")
                  for i in range(3)]
            si = [sml.tile([P, 4], F32, tag=f"l1si{i}", name=f"l1si{i}")
                  for i in range(3)]
            nc.vector.memset(sr[0][:, 0:2], 0.0)
            nc.vector.memset(sr[0][:, 2:4], 1.0)
            nc.vector.memset(sr[1][:, 0:2], 1.0)
            nc.vector.memset(sr[1][:, 2:4], 0.0)
            nc.vector.memset(si[0][:], 0.0)
            nc.vector.memset(si[1][:], 0.0)
            ta4r = sml.tile([P, 4], F32, tag="ta4r")
            tb4r = sml.tile([P, 4], F32, tag="tb4r")
            ta4i = sml.tile([P, 4], F32, tag="ta4i")
            tb4i = sml.tile([P, 4], F32, tag="tb4i")
            for t in range(16):
                p0, p1, p2 = t % 3, (t + 1) % 3, (t + 2) % 3
                ad = adup[:, 4 * t:4 * t + 4]
                tt(ta4r, ad, sr[p1][:], MULT)
                tt(ta4i, ad, si[p1][:], MULT)
                tt(tb4r, ta4r, si[p1][:], SUB)
                tt(tb4i, ta4i, sr[p1][:], ADD)
                tt(sr[p2][:], tb4r, sr[p0][:], SUB)
                tt(si[p2][:], tb4i, si[p0][:], SUB)
            srL, siL = sr[2], si[2]      # slot 17: m00 (cols 0:2), m01 (cols 2:4)
            srP, siP = sr[1], si[1]      # slot 16: m10, m11
            # Q cols: ri*8 + e*2 + dir, e in (00, 01, 10, 11)
            q = sml.tile([P, 16], F32, tag="qa")
            cp(q[:, 0:2], srL[:, 0:2])
            cp(q[:, 2:4], srL[:, 2:4])
            cp(q[:, 4:6], srP[:, 0:2])
            cp(q[:, 6:8], srP[:, 2:4])
            cp(q[:, 8:10], siL[:, 0:2])
            cp(q[:, 10:12], siL[:, 2:4])
            cp(q[:, 12:14], siP[:, 0:2])
            cp(q[:, 14:16], siP[:, 2:4])

            rn1 = sml.tile([P, 2], F32, tag="rn1")
            rn2 = sml.tile([P, 2], F32, tag="rn2")
            rn3 = sml.tile([P, 2], F32, tag="rn3")

            def renorm(qq):
                tt(rn1, qq[:, 0:2], qq[:, 0:2], MULT)
                tt(rn2, qq[:, 8:10], qq[:, 8:10], MULT)
                tt(rn3, rn1, rn2, ADD)
                nc.scalar.activation(out=rn1, in_=rn3, func=AF.Sqrt)
                nc.vector.reciprocal(out=rn2, in_=rn1)
                ts(qq[:, 0:16:2], qq[:, 0:16:2], rn2[:, 0:1], None, MULT)
                ts(qq[:, 1:16:2], qq[:, 1:16:2], rn2[:, 1:2], None, MULT)

            renorm(q)

            # packed L2: all four 2x2 complex entries per op via broadcast APs
            def patA(t, off):
                # cols off + 4i + c over (i, j, c); j broadcast
                base, c0 = (off // 8) * 8, off % 4
                b = t[:, base:base + 8].rearrange("p (i g) -> p i g", i=2)
                return b[:, :, c0:c0 + 2].unsqueeze(2).broadcast_to([P, 2, 2, 2])

            def patB(t, off):
                # cols off + 2j + c over (i, j, c); i broadcast
                b = t[:, off:off + 4].rearrange("p (j c) -> p j c", j=2)
                return b.unsqueeze(1).broadcast_to([P, 2, 2, 2])

            prt = [sml.tile([P, 8], F32, tag=f"prt{k}", name=f"prt{k}")
                   for k in range(8)]
            l2s = [sml.tile([P, 8], F32, tag=f"l2s{k}", name=f"l2s{k}")
                   for k in range(4)]
            PRODS = ((0, 0, 0, 0), (1, 8, 8, 0), (2, 2, 4, 0), (3, 10, 12, 0),
                     (4, 0, 8, 0), (5, 8, 0, 0), (6, 2, 12, 0), (7, 10, 4, 0))
            for i, s in enumerate((1, 2, 4, 8, 16, 32, 64)):
                psq = psp.tile([P, 16], F32, tag="mm")
                nc.tensor.matmul(out=psq, lhsT=sht[:, i * P:(i + 1) * P],
                                 rhs=q[:], start=True, stop=True)
                nc.vector.memset(psq[0:s, 0:2], 1.0)   # identity pad m00
                nc.vector.memset(psq[0:s, 6:8], 1.0)   # identity pad m11
                qn = sml.tile([P, 16], F32, tag=("qb" if i % 2 == 0 else "qa"))
                for k, qo, po, _ in PRODS:
                    tt(prt[k].rearrange("p (i j c) -> p i j c", i=2, j=2),
                       patA(q, qo), patB(psq, po), MULT)
                tt(l2s[0], prt[0], prt[1], SUB)
                tt(l2s[1], l2s[0], prt[2], ADD)
                tt(qn[:, 0:8], l2s[1], prt[3], SUB)
                tt(l2s[2], prt[4], prt[5], ADD)
                tt(l2s[3], l2s[2], prt[6], ADD)
                tt(qn[:, 8:16], l2s[3], prt[7], ADD)
                q = qn
                if i in (2, 5):
                    renorm(q)

            # L3: regen interior pairs from shifted chunk-start vectors
            psq1 = psp.tile([P, 16], F32, tag="mm")
            nc.tensor.matmul(out=psq1, lhsT=sht[:, 0:P], rhs=q[:],
                             start=True, stop=True)
            nc.vector.memset(psq1[0:1, 0:2], 1.0)      # chunk0 start x = 1
            s2r = sml.tile([P, 36], F32, tag="s2r")
            s2i = sml.tile([P, 36], F32, tag="s2i")
            cp(s2r[:, 0:2], psq1[:, 4:6])      # slot0 = ys = q10.re
            cp(s2i[:, 0:2], psq1[:, 12:14])
            cp(s2r[:, 2:4], psq1[:, 0:2])      # slot1 = xs = q00.re
            cp(s2i[:, 2:4], psq1[:, 8:10])
            l3ar = sml.tile([P, 2], F32, tag="l3ar")
            l3br = sml.tile([P, 2], F32, tag="l3br")
            l3ai = sml.tile([P, 2], F32, tag="l3ai")
            l3bi = sml.tile([P, 2], F32, tag="l3bi")
            for t in range(16):
                pv1 = slice(2 * t + 2, 2 * t + 4)
                pv0 = slice(2 * t, 2 * t + 2)
                ot_ = slice(2 * t + 4, 2 * t + 6)
                ad2 = adup[:, 4 * t:4 * t + 2]
                tt(l3ar, ad2, s2r[:, pv1], MULT)
                tt(l3ai, ad2, s2i[:, pv1], MULT)
                tt(l3br, l3ar, s2i[:, pv1], SUB)
                tt(l3bi, l3ai, s2r[:, pv1], ADD)
                tt(s2r[:, ot_], l3br, s2r[:, pv0], SUB)
                tt(s2i[:, ot_], l3bi, s2i[:, pv0], SUB)

            sfr = psp.tile([P, 36], F32, tag="mm")
            nc.tensor.matmul(out=sfr, lhsT=jmt[:], rhs=s2r[:], start=True, stop=True)
            sfi = psp.tile([P, 36], F32, tag="mm")
            nc.tensor.matmul(out=sfi, lhsT=jmt[:], rhs=s2i[:], start=True, stop=True)

            uxr, uxi = s2r[:, 4:36:2], s2i[:, 4:36:2]
            uyr, uyi = s2r[:, 2:34:2], s2i[:, 2:34:2]
            wxr, wxi = sfr[:, 35:3:-2], sfi[:, 35:3:-2]
            wyr, wyi = sfr[:, 33:1:-2], sfi[:, 33:1:-2]

            def ctile(tag):
                return sml.tile([P, 16], F32, tag=tag, name=tag)

            nr_, ni_ = ctile("nr"), ctile("ni")
            t1r, t1i = ctile("t1r"), ctile("t1i")
            t2r, t2i = ctile("t2r"), ctile("t2i")
            t3r, t3i = ctile("t3r"), ctile("t3i")
            drt, dit = ctile("drt"), ctile("dit")
            magt, invt = ctile("magt"), ctile("invt")
            gr, gi = ctile("gr"), ctile("gi")
            fts = [ctile(f"ft{k}") for k in range(10)]

            def cmul(or_, oi_, xr_, xi_, yr_, yi_, u0, u1, u2, u3):
                tt(u0, xr_, yr_, MULT)
                tt(u1, xi_, yi_, MULT)
                tt(u2, xr_, yi_, MULT)
                tt(u3, xi_, yr_, MULT)
                tt(or_, u0, u1, SUB)
                tt(oi_, u2, u3, ADD)

            cmul(nr_, ni_, uyr, uyi, wyr, wyi, *fts[0:4])    # num = Uy*Wy
            cmul(t1r, t1i, uxr, uxi, wyr, wyi, *fts[4:8])
            cmul(t2r, t2i, wxr, wxi, uyr, uyi, *fts[0:4])
            tt(fts[8], arf, nr_, MULT)               # t3 = a*num, a = arf + 1j
            tt(t3r, fts[8], ni_, SUB)
            tt(fts[9], arf, ni_, MULT)
            tt(t3i, fts[9], nr_, ADD)
            tt(fts[4], t1r, t2r, ADD)
            tt(drt, fts[4], t3r, SUB)
            tt(fts[5], t1i, t2i, ADD)
            tt(dit, fts[5], t3i, SUB)
            tt(fts[6], drt, drt, MULT)
            tt(fts[7], dit, dit, MULT)
            tt(magt, fts[6], fts[7], ADD)
            nc.vector.reciprocal(out=invt, in_=magt)
            tt(fts[0], nr_, drt, MULT)
            tt(fts[1], ni_, dit, MULT)
            tt(gr, fts[0], fts[1], ADD)
            tt(gr, gr, invt, MULT)
            tt(fts[2], ni_, drt, MULT)
            tt(fts[3], nr_, dit, MULT)
            tt(gi, fts[2], fts[3], SUB)
            tt(gi, gi, invt, MULT)

            # ================= stage F: G -> slot order =======================
            nc.sync.dma_start(out=grd[0:N], in_=gr[:])
            nc.sync.dma_start(out=gid[0:N], in_=gi[:])
            if debug:
                nc.sync.dma_start(out=grdbg[0:N], in_=gr[:])
                nc.sync.dma_start(out=gidbg[0:N], in_=gi[:])
            gg_rep = rep[:, 0:2 * N]
            nc.sync.dma_start(out=gg_rep[:, 0:N],
                              in_=grd[0:N].partition_broadcast(P))
            nc.sync.dma_start(out=gg_rep[:, N:2 * N],
                              in_=gid[0:N].partition_broadcast(P))
            grs = sml.tile([P, TT], F32, tag="grs")
            gis = sml.tile([P, TT], F32, tag="gis")
            ncop = (2 * TT + 7) // 8
            gts = [psp.tile([P, P], F32, tag="mm", name=f"gt{c}")
                   for c in range(ncop)]
            for c in range(ncop):
                midg = sml.tile([P, P], F32, tag=f"gmid{c}", name=f"gmid{c}")
                nc.gpsimd.indirect_copy(
                    out=midg[:], data=gg_rep, idxs=gidx[:, 8 * c:8 * (c + 1)],
                    i_know_ap_gather_is_preferred=True)
                nc.tensor.transpose(gts[c][:], midg[:], idt[:])
            for k0 in range(0, 2 * TT, 8):
                c = k0 // 8
                nk = min(8, 2 * TT - k0)
                # tasks k0..k0+nk-1 map to columns 0,16,... of gts[c]
                k = k0
                while k < k0 + nk:
                    comp, tpos = k // TT, k % TT
                    run = 1
                    while (k + run < k0 + nk and (k + run) // TT == comp):
                        run += 1
                    dst = grs if comp == 0 else gis
                    g0 = (k - k0)
                    cp(dst[:, tpos:tpos + run],
                       gts[c][:, g0 * 16:(g0 + run) * 16:16])
                    k += run
            if debug:
                nc.sync.dma_start(out=grsdbg[:], in_=grs)
                nc.sync.dma_start(out=gisdbg[:], in_=gis)

            # ================= stage G: final combine =========================
            for tb in range(TT):
                acc = rot.tile([P, D], F32, tag="wrk")
                stt(acc, w0t, grs[:, tb:tb + 1], outbt, MULT, ADD)
                acc2 = rot.tile([P, D], F32, tag="wrk")
                stt(acc2, w1ot, gis[:, tb:tb + 1], acc, MULT, ADD)
                ott = rot.tile([P, D], F32, tag="wrk")
                tt(ott, acc2, moe[:, tb * D:(tb + 1) * D], ADD)
                nc.sync.dma_start(out=out[tb * P:(tb + 1) * P, :], in_=ott)
                if debug:
                    nc.sync.dma_start(out=moedbg[tb * P:(tb + 1) * P, :],
                                      in_=moe[:, tb * D:(tb + 1) * D])
    nc.finalize()
    return nc


def _route(inputs):
    """Replicate the reference's routing decision exactly (jax CPU fp32)."""
    import jax
    import jax.numpy as jnp
    cpu = jax.devices("cpu")[0]
    f = np.float32
    with jax.default_device(cpu):
        x = jnp.asarray(np.asarray(inputs["x"], f))
        gam = jnp.asarray(np.asarray(inputs["ln_gamma"], f))
        bet = jnp.asarray(np.asarray(inputs["ln_beta"], f))
        gw = jnp.asarray(np.asarray(inputs["gate_w"], f))
        gb = jnp.asarray(np.asarray(inputs["gate_b"], f))
        mu = x.mean(-1, keepdims=True)
        var = x.var(-1, keepdims=True)
        xn = (x - mu) / jnp.sqrt(var + 1e-5) * gam + bet
        probs = jax.nn.softmax(xn @ gw + gb, axis=-1)
        idx = np.asarray(jnp.argmax(probs, axis=-1))
    return idx  # (B, N) int


def _prep_inputs(inputs):
    import ml_dtypes
    f = np.float32
    bf16 = ml_dtypes.bfloat16
    x = np.ascontiguousarray(np.asarray(inputs["x"], f))          # (B, N, D)
    gamma = np.asarray(inputs["ln_gamma"], f)
    beta = np.asarray(inputs["ln_beta"], f)
    gate_w = np.asarray(inputs["gate_w"], f)
    gate_b = np.asarray(inputs["gate_b"], f)
    w1 = np.asarray(inputs["w1"], f)
    b1 = np.asarray(inputs["b1"], f)
    w2 = np.asarray(inputs["w2"], f)
    b2 = np.asarray(inputs["b2"], f)
    proj_w = np.asarray(inputs["proj_w"], f)[:, 0]
    out_w = np.asarray(inputs["out_w"], f)
    out_b = np.asarray(inputs["out_b"], f)
    bk = f(np.asarray(inputs["bk_scale"], f).reshape(-1)[0])
    proj_b_imm = float(np.asarray(inputs["proj_b"], f).reshape(-1)[0])

    idx = _route(inputs)

    # slot assignment: core c = 2b + h handles row b, experts {2h, 2h+1}
    toklists = {}
    capt = 1
    for b in range(B):
        for e in range(E):
            toks = np.nonzero(idx[b] == e)[0]
            toklists[(b, e)] = toks
            capt = max(capt, (len(toks) + P - 1) // P)
    TT = 2 * capt
    S = TT * P
    S2 = capt * P

    def bcast(v, w):
        return np.ascontiguousarray(np.broadcast_to(v.astype(f), (P, w)))

    common = dict(
        gamT=np.ascontiguousarray(gamma.reshape(4, P).T),
        betT=np.ascontiguousarray(beta.reshape(4, P).T),
        gwsb=np.ascontiguousarray(
            gate_w.reshape(4, P, E).transpose(1, 0, 2).reshape(P, 16)
        ).astype(bf16),
        gatebb=bcast(gate_b, E),
        projwb=bcast(proj_w, D),
        w0b=bcast(out_w[0] * bk, D),
        w1ob=bcast(out_w[1] * bk, D),
        outbb=bcast(out_b * bk, D),
        shmat=np.ascontiguousarray(np.concatenate(
            [np.eye(P, k=s, dtype=f) for s in (1, 2, 4, 8, 16, 32, 64)], axis=1)),
        jmat=np.ascontiguousarray(np.eye(P, dtype=f)[::-1]),
        idm=np.eye(P, dtype=f),
    )
    in_maps = []
    slotmaps = []
    for c in range(NCORE):
        b, h = c // 2, c % 2
        m = dict(common)
        xs = np.zeros((S, D), f)
        slot_tok = np.full(S, -1, np.int64)
        for el in range(2):
            toks = toklists[(b, 2 * h + el)]
            xs[el * S2: el * S2 + len(toks)] = x[b, toks]
            slot_tok[el * S2: el * S2 + len(toks)] = toks
        m["xs"] = np.ascontiguousarray(xs.astype(bf16))
        m["w1s"] = np.ascontiguousarray(
            w1[2 * h:2 * h + 2].reshape(2 * D, H).astype(bf16))
        m["w2s"] = np.ascontiguousarray(
            w2[2 * h:2 * h + 2].reshape(2 * H, D).astype(bf16))
        b1v = np.zeros((P, 32), f)
        for el in range(2):
            for hcc in range(16):
                b1v[:, el * 16 + hcc] = b1[2 * h + el, hcc * P:(hcc + 1) * P]
        m["b1s"] = b1v
        m["b2s"] = bcast(b2[2 * h:2 * h + 2].reshape(2 * D), 2 * D)

        # slot position within [vrowA | vrowB] of each original token n
        pairpos = np.zeros(N, np.int64)
        for h2 in range(2):
            for el in range(2):
                toks = toklists[(b, 2 * h2 + el)]
                pairpos[toks] = el * 2 * S2 + h2 * S2 + np.arange(len(toks))
        vidx = np.zeros((P, 16), np.uint16)
        for g in range(8):
            for k in range(P):
                vidx[16 * g + (k % 16), k // 16] = pairpos[16 * k + g]
                vidx[16 * g + (k % 16), 8 + k // 16] = pairpos[16 * k + 8 + g]
        m["vidxd"] = vidx

        gidxv = np.zeros((P, 24), np.uint16)
        for k in range(2 * TT):
            comp, tpos = k // TT, k % TT
            cpy, g = k // 8, k % 8
            for kk in range(P):
                n_s = slot_tok[tpos * P + kk]
                val = (int(n_s) if n_s >= 0 else 0) + comp * N
                gidxv[16 * g + (kk % 16), 8 * cpy + kk // 16] = val
        m["gidxd"] = gidxv
        in_maps.append(m)
        slotmaps.append(slot_tok)
    return in_maps, slotmaps, proj_b_imm, capt


def _run(inputs, debug=False, trace=False, trace_cores=None):
    in_maps, slotmaps, proj_b_imm, capt = _prep_inputs(inputs)
    nc = build(proj_b_imm, capt, debug=debug)
    kw = {}
    if trace_cores is not None:
        kw["trace_cores"] = trace_cores
    res = run_bass_kernel_spmd(nc, in_maps, core_ids=list(range(NCORE)),
                               trace=trace, **kw)
    out = np.zeros((B, N, D), np.float32)
    for c in range(NCORE):
        b = c // 2
        rows = np.asarray(res.results[c]["out"])
        st = slotmaps[c]
        real = st >= 0
        out[b, st[real]] = rows[real]
    return out, res


def kernel(**inputs):
    out, _ = _run(inputs)
    return out
